# revision 7
# baseline (speedup 1.0000x reference)
"""
W8A8 quantized linear (dynamic per-token int8 activation quant + int8 weight,
fp32 dequant) on 8 Trainium2 NeuronCores.

Reference semantics (per token m, output channel n):
    absmax[m] = max_k |x[m,k]|            (fp32)
    scale[m]  = max(absmax[m]/127, 1e-8)
    q[m,k]    = round(x[m,k] / scale[m])  in [-127, 127]   (round-half-even)
    y[m,n]    = (sum_k q[m,k] * w[n,k]) * scale[m] * wscale[n]   -> fp16

Sharding: data-parallel over tokens (8192 tokens -> 1024/core); weight is
replicated. Host pre-transposes the weight to [K, N] bf16 (exact) AND
pre-arranges it n-slice-major/partition-contiguous ([NS, 128, KT*NSL]) so each
512-wide slice streams as 128 x 32KB descriptors (full HBM bandwidth, cheap
ring injection).

Device kernel (engine assignment tuned from NTFF traces):
  warmup: dummy matmuls on zeroed SBUF from t~0 keep the PE's HAM clock-gate
    at 2.4 GHz and cover the quant-pipeline fill latency.
  phase A/B (per 128-token tile, chunked by 1024 columns): x chunk DMAs on
    the SCALAR queue rings; per-chunk absmax partials + combine + scale/recip
    on DVE; quantize chunks on GPSIMD (x*inv + 1.5*2^23 fp32 magic-round)
    then ACT (- 1.5*2^23 -> bf16); per-chunk DMA-xbar transposes on the SYNC
    queue into qT[p, mt, kt, m].
  phase C: wt0+wt1 prefetched up front; the first two n-slices are processed
    interleaved per m-tile ((0,mt),(1,mt)) so each quantized m-tile unlocks
    2x the PE work during pipeline fill; then n-slices 2..7 run m-inner.
    KT=32 matmuls accumulate per (m-tile, n-slice) into one PSUM bank;
    dequant on evacuation is ONE fused DVE op:
    scalar_tensor_tensor ot = (psum * scale[m]) * wscale[n] -> fp16,
    then y DMA on the SCALAR queue.
"""

import os
import numpy as np
import ml_dtypes
from contextlib import ExitStack

import concourse.bass as bass
import concourse.mybir as mybir
import concourse.tile as tile
from concourse import bacc

QMAX = 127.0
MAGIC = 1.5 * 2**23  # fp32 round-to-nearest-even trick for |v| < 2^22

F16 = mybir.dt.float16
BF16 = mybir.dt.bfloat16
F32 = mybir.dt.float32


def build_nc(M=1024, K=4096, N=4096, NSL=512, QCH=1024, WARM=60,
             do_quant=True, do_mm=True, reps=1):
    """One-core program; run SPMD on 8 cores with different token shards."""
    nc = bacc.Bacc()
    MT, KT, NS = M // 128, K // 128, N // NSL
    QCH = min(QCH, K)
    NQ = K // QCH          # quant chunks per m-tile
    KTC = QCH // 128       # kt tiles per quant chunk

    x = nc.declare_dram_parameter("x", [M, K], F16, isOutput=False)
    # weight, n-slice-major, partition-contiguous: [NS, 128, KT*NSL]
    wTr = nc.declare_dram_parameter("wTr", [NS, 128, KT * NSL], BF16,
                                    isOutput=False)
    wsb = nc.declare_dram_parameter("wsb", [128, N], F16, isOutput=False)
    y = nc.declare_dram_parameter("y", [M, N], F16, isOutput=True)

    with tile.TileContext(nc) as tc, ExitStack() as ctx:
      pers = ctx.enter_context(tc.tile_pool(name="pers", bufs=1))
      qpool = ctx.enter_context(tc.tile_pool(name="qt", bufs=1))
      xpool = ctx.enter_context(tc.tile_pool(name="xa", bufs=2))
      tpool = ctx.enter_context(tc.tile_pool(name="tmpq", bufs=2))
      qnat = ctx.enter_context(tc.tile_pool(name="qnat", bufs=2))
      wpool = ctx.enter_context(tc.tile_pool(name="wt", bufs=2))
      psum = ctx.enter_context(tc.tile_pool(name="psum", bufs=6, space="PSUM"))
      opool = ctx.enter_context(tc.tile_pool(name="out", bufs=3))
      for rep in range(reps):
        if rep > 0:
            tc.strict_bb_all_engine_barrier()

        # -- prefetch: first TWO weight slices (sync queue), wsb (scalar) --
        wts = {}
        wts[0] = wpool.tile([128, KT, NSL], BF16, tag="wt", name="wt0")
        wts[1] = wpool.tile([128, KT, NSL], BF16, tag="wt", name="wt1")
        if do_mm:
            nc.sync.dma_start(wts[0][:], wTr[0])
            nc.sync.dma_start(wts[1][:], wTr[1])
        wsb_sb = pers.tile([128, N], F16)
        nc.scalar.dma_start(wsb_sb[:], wsb[:])

        # -- PE warmup: dummy matmuls on zeroed SBUF from t~0 so HAM is at
        #    2.4 GHz (and stays there) when the first real matmul issues --
        if do_mm and WARM > 0:
            warm = pers.tile([128, NSL], BF16)
            nc.vector.memset(warm[:], 0.0)
            wps = psum.tile([128, NSL], F32, tag="pt")
            for _ in range(WARM):
                nc.tensor.matmul(wps[:], warm[:, 0:128], warm[:],
                                 start=True, stop=True)

        am4 = pers.tile([128, MT, NQ], F32)
        am = pers.tile([128, MT], F32)
        scales = pers.tile([128, MT], F32)
        invs = pers.tile([128, MT], F32)
        # m-tile-major so each m-tile's [KT,128] block is contiguous; chunked
        # xbar transposes write qT[p, mt, kt, m] = q[mt*128+m, kt*128+p]
        qT = qpool.tile([128, MT, KT, 128], BF16)

        if not do_quant:
            nc.vector.memset(scales[:], 1.0)
            if do_mm:
                nc.vector.memset(qT[:], 1.0)
        # ---- phase A/B: per m-tile absmax, scales, quantize, transpose ----
        for mt in range(MT if do_quant else 0):
            xa = xpool.tile([128, NQ, QCH], F16, tag="xa")
            for kc in range(NQ):
                nc.scalar.dma_start(
                    xa[:, kc],
                    x[mt * 128 : (mt + 1) * 128, kc * QCH : (kc + 1) * QCH],
                )
                # per-chunk absmax partial (pipelines with the x DMA)
                nc.vector.tensor_reduce(
                    am4[:, mt, kc : kc + 1],
                    xa[:, kc],
                    axis=mybir.AxisListType.X,
                    op=mybir.AluOpType.max,
                    apply_absolute_value=True,
                )
            nc.vector.tensor_reduce(
                am[:, mt : mt + 1],
                am4[:, mt],
                axis=mybir.AxisListType.X,
                op=mybir.AluOpType.max,
            )
            # scale = max(absmax/127, 1e-8); inv = 1/scale
            nc.vector.tensor_scalar(
                scales[:, mt : mt + 1],
                am[:, mt : mt + 1],
                1.0 / QMAX,
                1e-8,
                mybir.AluOpType.mult,
                mybir.AluOpType.max,
            )
            nc.vector.reciprocal(invs[:, mt : mt + 1], scales[:, mt : mt + 1])

            qn = qnat.tile([128, K], BF16, tag="qn")
            for kc in range(NQ):
                sl = slice(kc * QCH, (kc + 1) * QCH)
                tmpq = tpool.tile([128, QCH], F32, tag="tmpq")
                # tmpq = x*inv + MAGIC  (fp32; rounds to integer at +MAGIC)
                nc.gpsimd.tensor_scalar(
                    tmpq[:],
                    xa[:, kc],
                    invs[:, mt : mt + 1],
                    MAGIC,
                    mybir.AluOpType.mult,
                    mybir.AluOpType.add,
                )
                # qn = tmpq - MAGIC  (exact; integer-valued, exact in bf16)
                nc.scalar.activation(
                    qn[:, sl],
                    tmpq[:],
                    mybir.ActivationFunctionType.Copy,
                    bias=-MAGIC,
                )
                # chunk xbar transpose: [128m, 1024k] -> [128k, KTC, 128m]
                nc.sync.dma_start_transpose(
                    qT[:, mt, kc * KTC : (kc + 1) * KTC], qn[:, sl]
                )

        if not do_mm:
            ot0 = opool.tile([128, NSL], F16, tag="ot")
            nc.vector.memset(ot0[:], 0.0)
            nc.sync.dma_start(y[0:128, 0:NSL], ot0[:])
        # ---- phase C: matmul + fused dequant ----
        # First two n-slices interleaved per m-tile (both weight slices are
        # resident): each new qT tile unlocks 2 chains during pipeline fill.
        ILV = 4  # m-tiles processed (ns0,ns1)-interleaved
        sched = []
        for mt in range(ILV):
            sched += [(0, mt), (1, mt)]
        sched += [(0, mt) for mt in range(ILV, MT)]
        sched += [(1, mt) for mt in range(ILV, MT)]
        for ns in range(2, NS):
            sched += [(ns, mt) for mt in range(MT)]

        for ns, mt in (sched if do_mm else []):
            nsl = slice(ns * NSL, (ns + 1) * NSL)
            if ns not in wts:
                wt = wpool.tile([128, KT, NSL], BF16, tag="wt", name=f"wt{ns}")
                nc.sync.dma_start(wt[:], wTr[ns])
                wts[ns] = wt
            wt = wts[ns]
            pt = psum.tile([128, NSL], F32, tag="pt")
            for kt in range(KT):
                nc.tensor.matmul(
                    pt[:],
                    qT[:, mt, kt, :],
                    wt[:, kt, :],
                    start=(kt == 0),
                    stop=(kt == KT - 1),
                )
            # ot = (psum * scale[m]) * wscale[n] -> fp16, one DVE op
            ot = opool.tile([128, NSL], F16, tag="ot")
            nc.vector.scalar_tensor_tensor(
                ot[:],
                pt[:],
                scales[:, mt : mt + 1],
                wsb_sb[:, nsl],
                mybir.AluOpType.mult,
                mybir.AluOpType.mult,
            )
            nc.scalar.dma_start(y[mt * 128 : (mt + 1) * 128, nsl], ot[:])

    nc.finalize()  # Bacc.compile(): reg alloc, wait-splitting, event sems
    return nc


def prep_inputs(x, weight, weight_scales, n_cores=8, NSL=512):
    """Host-side shard/layout prep. Returns (in_maps, out_assembler)."""
    B, S, D_in = x.shape
    D_out = weight.shape[0]
    M_total = B * S
    Mc = M_total // n_cores
    KT = D_in // 128
    NS = D_out // NSL

    xf = np.ascontiguousarray(np.asarray(x).reshape(M_total, D_in))
    w = np.asarray(weight)
    wT = w.T.astype(np.float32).astype(ml_dtypes.bfloat16)  # [K, N] exact
    # n-slice-major, partition-contiguous: wTr[ns, p, kt*NSL + j] =
    # wT[kt*128 + p, ns*NSL + j]
    wTr = np.ascontiguousarray(
        wT.reshape(KT, 128, NS, NSL).transpose(2, 1, 0, 3).reshape(
            NS, 128, KT * NSL
        )
    )
    ws = np.asarray(weight_scales).astype(np.float16)
    wsb = np.ascontiguousarray(np.broadcast_to(ws[None, :], (128, D_out)))

    in_maps = [
        {"x": xf[c * Mc : (c + 1) * Mc], "wTr": wTr, "wsb": wsb}
        for c in range(n_cores)
    ]

    def assemble(results):
        return np.concatenate(
            [np.asarray(results[c]["y"]) for c in range(n_cores)], axis=0
        ).reshape(B, S, D_out).astype(np.float16)

    return in_maps, assemble


def kernel(x, weight, weight_scales):
    from concourse.bass_utils import run_bass_kernel_spmd

    n_cores = 8
    B, S, D_in = x.shape
    D_out = weight.shape[0]
    Mc = (B * S) // n_cores

    nc = build_nc(M=Mc, K=D_in, N=D_out)
    in_maps, assemble = prep_inputs(x, weight, weight_scales, n_cores)
    res = run_bass_kernel_spmd(nc, in_maps, list(range(n_cores)))
    return assemble(res.results)


if __name__ == "__main__":
    np.random.seed(0)
    x = np.random.randn(4, 2048, 4096).astype(np.float16)
    w = np.random.randint(-127, 127, (4096, 4096)).astype(np.int8)
    ws = (np.random.rand(4096).astype(np.float32) * 0.01 + 1e-4).astype(np.float16)
    y = kernel(x, w, ws)
    print(y.shape, y.dtype)


# revision 12
# speedup vs baseline: 1.0855x; 1.0855x over previous
"""
W8A8 quantized linear (dynamic per-token int8 activation quant + int8 weight,
fp32 dequant) on 8 Trainium2 NeuronCores.

Reference semantics (per token m, output channel n):
    absmax[m] = max_k |x[m,k]|            (fp32)
    scale[m]  = max(absmax[m]/127, 1e-8)
    q[m,k]    = round(x[m,k] / scale[m])  in [-127, 127]   (round-half-even)
    y[m,n]    = (sum_k q[m,k] * w[n,k]) * scale[m] * wscale[n]   -> fp16

Sharding: data-parallel over tokens (8192 tokens -> 1024/core); weight is
replicated. Host pre-transposes the weight to [K, N] bf16 (exact) AND
pre-arranges it n-slice-major/partition-contiguous ([NS, 128, KT*NSL]) so each
512-wide slice streams as 128 x 32KB descriptors (full HBM bandwidth, cheap
ring injection).

Device kernel (engine assignment tuned from NTFF traces):
  warmup: dummy matmuls on zeroed SBUF from t~0 keep the PE's HAM clock-gate
    at 2.4 GHz and cover the quant-pipeline fill latency.
  phase A/B (per 128-token tile, chunked by 1024 columns): x chunk DMAs on
    the SCALAR queue rings; per-chunk absmax partials + combine + scale/recip
    on DVE; quantize chunks on GPSIMD (x*inv + 1.5*2^23 fp32 magic-round)
    then ACT (- 1.5*2^23 -> bf16); per-chunk DMA-xbar transposes on the SYNC
    queue into qT[p, mt, kt, m].
  phase C: wt0+wt1 prefetched up front; the first two n-slices are processed
    interleaved per m-tile ((0,mt),(1,mt)) so each quantized m-tile unlocks
    2x the PE work during pipeline fill; then n-slices 2..7 run m-inner.
    KT=32 matmuls accumulate per (m-tile, n-slice) into one PSUM bank;
    dequant on evacuation is ONE fused DVE op:
    scalar_tensor_tensor ot = (psum * scale[m]) * wscale[n] -> fp16,
    then y DMA on the SCALAR queue.
"""

import os
import numpy as np
import ml_dtypes
from contextlib import ExitStack

import concourse.bass as bass
import concourse.mybir as mybir
import concourse.tile as tile
from concourse import bacc

QMAX = 127.0
MAGIC = 1.5 * 2**23  # fp32 round-to-nearest-even trick for |v| < 2^22

F16 = mybir.dt.float16
BF16 = mybir.dt.bfloat16
F32 = mybir.dt.float32


def build_nc(M=1024, K=4096, N=4096, NSL=512, QCH=1024, WARM=60,
             do_quant=True, do_mm=True, reps=1):
    """One-core program; run SPMD on 8 cores with different token shards."""
    nc = bacc.Bacc()
    MT, KT, NS = M // 128, K // 128, N // NSL
    QCH = min(QCH, K)
    NQ = K // QCH          # quant chunks per m-tile
    KTC = QCH // 128       # kt tiles per quant chunk

    x = nc.declare_dram_parameter("x", [M, K], F16, isOutput=False)
    # weight, n-slice-major, partition-contiguous: [NS, 128, KT*NSL]
    wTr = nc.declare_dram_parameter("wTr", [NS, 128, KT * NSL], BF16,
                                    isOutput=False)
    wsb = nc.declare_dram_parameter("wsb", [128, N], F16, isOutput=False)
    y = nc.declare_dram_parameter("y", [M, N], F16, isOutput=True)

    with tile.TileContext(nc) as tc, ExitStack() as ctx:
      pers = ctx.enter_context(tc.tile_pool(name="pers", bufs=1))
      qpool = ctx.enter_context(tc.tile_pool(name="qt", bufs=1))
      xpool = ctx.enter_context(tc.tile_pool(name="xa", bufs=4))
      tpool = ctx.enter_context(tc.tile_pool(name="tmpq", bufs=2))
      qnat = ctx.enter_context(tc.tile_pool(name="qnat", bufs=2))
      wpool = ctx.enter_context(tc.tile_pool(name="wt", bufs=2))
      psum = ctx.enter_context(tc.tile_pool(name="psum", bufs=6, space="PSUM"))
      opool = ctx.enter_context(tc.tile_pool(name="out", bufs=3))
      for rep in range(reps):
        if rep > 0:
            tc.strict_bb_all_engine_barrier()

        # -- software-pipelined x loads (bufs=4): x0 leads the sync ring
        #    ahead of the weight slices; odd tiles ride the scalar ring --
        xas = [None] * MT

        def load_x(mt, eng):
            xas[mt] = xpool.tile([128, K], F16, tag="xa", name=f"xa{mt}")
            eng.dma_start(xas[mt][:], x[mt * 128 : (mt + 1) * 128, :])

        if do_quant:
            load_x(0, nc.sync)
        wts = {}
        wts[0] = wpool.tile([128, KT, NSL], BF16, tag="wt", name="wt0")
        wts[1] = wpool.tile([128, KT, NSL], BF16, tag="wt", name="wt1")
        if do_mm:
            nc.sync.dma_start(wts[0][:], wTr[0])
            nc.sync.dma_start(wts[1][:], wTr[1])
        wsb_sb = pers.tile([128, N], F16)
        if do_quant:
            load_x(1, nc.scalar)
        nc.scalar.dma_start(wsb_sb[:], wsb[:])
        if do_quant:
            load_x(2, nc.sync)
            if MT > 3:
                load_x(3, nc.scalar)

        # -- PE warmup: dummy matmuls on zeroed SBUF from t~0 so HAM is at
        #    2.4 GHz (and stays there) when the first real matmul issues --
        if do_mm and WARM > 0:
            warm = pers.tile([128, NSL], BF16)
            nc.vector.memset(warm[:], 0.0)
            wps = psum.tile([128, NSL], F32, tag="pt")
            for _ in range(WARM):
                nc.tensor.matmul(wps[:], warm[:, 0:128], warm[:],
                                 start=True, stop=True)

        am = pers.tile([128, MT], F32)
        scales = pers.tile([128, MT], F32)
        invs = pers.tile([128, MT], F32)
        # m-tile-major so each m-tile's [KT,128] block is contiguous; chunked
        # xbar transposes write qT[p, mt, kt, m] = q[mt*128+m, kt*128+p]
        qT = qpool.tile([128, MT, KT, 128], BF16)

        if not do_quant:
            nc.vector.memset(scales[:], 1.0)
            if do_mm:
                nc.vector.memset(qT[:], 1.0)
        # ---- phase A/B: per m-tile absmax, scales, quantize, transpose ----
        for mt in range(MT if do_quant else 0):
            xa = xas[mt]
            nc.vector.tensor_reduce(
                am[:, mt : mt + 1],
                xa[:],
                axis=mybir.AxisListType.X,
                op=mybir.AluOpType.max,
                apply_absolute_value=True,
            )
            # scale = max(absmax/127, 1e-8); inv = 1/scale
            nc.vector.tensor_scalar(
                scales[:, mt : mt + 1],
                am[:, mt : mt + 1],
                1.0 / QMAX,
                1e-8,
                mybir.AluOpType.mult,
                mybir.AluOpType.max,
            )
            nc.vector.reciprocal(invs[:, mt : mt + 1], scales[:, mt : mt + 1])

            qn = qnat.tile([128, K], BF16, tag="qn")
            for kc in range(NQ):
                sl = slice(kc * QCH, (kc + 1) * QCH)
                tmpq = tpool.tile([128, QCH], F32, tag="tmpq")
                # tmpq = x*inv + MAGIC  (fp32; rounds to integer at +MAGIC)
                nc.gpsimd.tensor_scalar(
                    tmpq[:],
                    xa[:, sl],
                    invs[:, mt : mt + 1],
                    MAGIC,
                    mybir.AluOpType.mult,
                    mybir.AluOpType.add,
                )
                # qn = tmpq - MAGIC  (exact; integer-valued, exact in bf16)
                nc.scalar.activation(
                    qn[:, sl],
                    tmpq[:],
                    mybir.ActivationFunctionType.Copy,
                    bias=-MAGIC,
                )
            # next x tile load rides whichever ring frees first; emitted
            # here so its buffer-free wait never blocks a transpose inject
            if mt + 4 < MT:
                load_x(mt + 4, nc.sync if (mt % 2 == 0) else nc.scalar)
            # one xbar transpose for the whole m-tile: [128m, 4096k] ->
            # [128k-part, KT, 128m] (contiguous dst block)
            nc.sync.dma_start_transpose(qT[:, mt], qn[:, :])

        if not do_mm:
            ot0 = opool.tile([128, NSL], F16, tag="ot")
            nc.vector.memset(ot0[:], 0.0)
            nc.sync.dma_start(y[0:128, 0:NSL], ot0[:])
        # ---- phase C: matmul + fused dequant ----
        # First two n-slices interleaved per m-tile (both weight slices are
        # resident): each new qT tile unlocks 2 chains during pipeline fill.
        ILV = 4  # m-tiles processed (ns0,ns1)-interleaved
        sched = []
        for mt in range(ILV):
            sched += [(0, mt), (1, mt)]
        sched += [(0, mt) for mt in range(ILV, MT)]
        sched += [(1, mt) for mt in range(ILV, MT)]
        for ns in range(2, NS):
            sched += [(ns, mt) for mt in range(MT)]

        for ns, mt in (sched if do_mm else []):
            nsl = slice(ns * NSL, (ns + 1) * NSL)
            if ns not in wts:
                wt = wpool.tile([128, KT, NSL], BF16, tag="wt", name=f"wt{ns}")
                nc.sync.dma_start(wt[:], wTr[ns])
                wts[ns] = wt
            wt = wts[ns]
            pt = psum.tile([128, NSL], F32, tag="pt")
            for kt in range(KT):
                nc.tensor.matmul(
                    pt[:],
                    qT[:, mt, kt, :],
                    wt[:, kt, :],
                    start=(kt == 0),
                    stop=(kt == KT - 1),
                )
            # ot = (psum * scale[m]) * wscale[n] -> fp16, one DVE op
            ot = opool.tile([128, NSL], F16, tag="ot")
            nc.vector.scalar_tensor_tensor(
                ot[:],
                pt[:],
                scales[:, mt : mt + 1],
                wsb_sb[:, nsl],
                mybir.AluOpType.mult,
                mybir.AluOpType.mult,
            )
            nc.scalar.dma_start(y[mt * 128 : (mt + 1) * 128, nsl], ot[:])

    nc.finalize()  # Bacc.compile(): reg alloc, wait-splitting, event sems
    return nc


def prep_inputs(x, weight, weight_scales, n_cores=8, NSL=512):
    """Host-side shard/layout prep. Returns (in_maps, out_assembler)."""
    B, S, D_in = x.shape
    D_out = weight.shape[0]
    M_total = B * S
    Mc = M_total // n_cores
    KT = D_in // 128
    NS = D_out // NSL

    xf = np.ascontiguousarray(np.asarray(x).reshape(M_total, D_in))
    w = np.asarray(weight)
    wT = w.T.astype(np.float32).astype(ml_dtypes.bfloat16)  # [K, N] exact
    # n-slice-major, partition-contiguous: wTr[ns, p, kt*NSL + j] =
    # wT[kt*128 + p, ns*NSL + j]
    wTr = np.ascontiguousarray(
        wT.reshape(KT, 128, NS, NSL).transpose(2, 1, 0, 3).reshape(
            NS, 128, KT * NSL
        )
    )
    ws = np.asarray(weight_scales).astype(np.float16)
    wsb = np.ascontiguousarray(np.broadcast_to(ws[None, :], (128, D_out)))

    in_maps = [
        {"x": xf[c * Mc : (c + 1) * Mc], "wTr": wTr, "wsb": wsb}
        for c in range(n_cores)
    ]

    def assemble(results):
        return np.concatenate(
            [np.asarray(results[c]["y"]) for c in range(n_cores)], axis=0
        ).reshape(B, S, D_out).astype(np.float16)

    return in_maps, assemble


def kernel(x, weight, weight_scales):
    from concourse.bass_utils import run_bass_kernel_spmd

    n_cores = 8
    B, S, D_in = x.shape
    D_out = weight.shape[0]
    Mc = (B * S) // n_cores

    nc = build_nc(M=Mc, K=D_in, N=D_out)
    in_maps, assemble = prep_inputs(x, weight, weight_scales, n_cores)
    res = run_bass_kernel_spmd(nc, in_maps, list(range(n_cores)))
    return assemble(res.results)


if __name__ == "__main__":
    np.random.seed(0)
    x = np.random.randn(4, 2048, 4096).astype(np.float16)
    w = np.random.randint(-127, 127, (4096, 4096)).astype(np.int8)
    ws = (np.random.rand(4096).astype(np.float32) * 0.01 + 1e-4).astype(np.float16)
    y = kernel(x, w, ws)
    print(y.shape, y.dtype)


# revision 16
# speedup vs baseline: 1.1089x; 1.0215x over previous
"""
W8A8 quantized linear (dynamic per-token int8 activation quant + int8 weight,
fp32 dequant) on 8 Trainium2 NeuronCores.

Reference semantics (per token m, output channel n):
    absmax[m] = max_k |x[m,k]|            (fp32)
    scale[m]  = max(absmax[m]/127, 1e-8)
    q[m,k]    = round(x[m,k] / scale[m])  in [-127, 127]   (round-half-even)
    y[m,n]    = (sum_k q[m,k] * w[n,k]) * scale[m] * wscale[n]   -> fp16

Sharding: data-parallel over tokens (8192 tokens -> 1024/core); weight is
replicated. Host pre-transposes the weight to [K, N] bf16 (exact) AND
pre-arranges it n-slice-major/partition-contiguous ([NS, 128, KT*NSL]) so each
512-wide slice streams as 128 x 32KB descriptors (full HBM bandwidth, cheap
ring injection).

Device kernel (engine assignment tuned from NTFF traces):
  warmup: dummy matmuls on zeroed SBUF from t~0 keep the PE's HAM clock-gate
    at 2.4 GHz and cover the quant-pipeline fill latency.
  phase A/B (per 128-token tile, chunked by 1024 columns): x chunk DMAs on
    the SCALAR queue rings; per-chunk absmax partials + combine + scale/recip
    on DVE; quantize chunks on GPSIMD (x*inv + 1.5*2^23 fp32 magic-round)
    then ACT (- 1.5*2^23 -> bf16); per-chunk DMA-xbar transposes on the SYNC
    queue into qT[p, mt, kt, m].
  phase C: wt0+wt1 prefetched up front; the first two n-slices are processed
    interleaved per m-tile ((0,mt),(1,mt)) so each quantized m-tile unlocks
    2x the PE work during pipeline fill; then n-slices 2..7 run m-inner.
    KT=32 matmuls accumulate per (m-tile, n-slice) into one PSUM bank;
    dequant on evacuation is ONE fused DVE op:
    scalar_tensor_tensor ot = (psum * scale[m]) * wscale[n] -> fp16,
    then y DMA on the SCALAR queue.
"""

import os
import numpy as np
import ml_dtypes
from contextlib import ExitStack

import concourse.bass as bass
import concourse.mybir as mybir
import concourse.tile as tile
from concourse import bacc

QMAX = 127.0
MAGIC = 1.5 * 2**23  # fp32 round-to-nearest-even trick for |v| < 2^22

F16 = mybir.dt.float16
BF16 = mybir.dt.bfloat16
F32 = mybir.dt.float32


def build_nc(M=1024, K=4096, N=4096, NSL=512, QCH=1024, WARM=60,
             do_quant=True, do_mm=True, reps=1):
    """One-core program; run SPMD on 8 cores with different token shards."""
    nc = bacc.Bacc()
    MT, KT, NS = M // 128, K // 128, N // NSL
    QCH = min(QCH, K)
    NQ = K // QCH          # quant chunks per m-tile
    KTC = QCH // 128       # kt tiles per quant chunk

    x = nc.declare_dram_parameter("x", [M, K], F16, isOutput=False)
    # weight, n-slice-major, partition-contiguous: [NS, 128, KT*NSL]
    wTr = nc.declare_dram_parameter("wTr", [NS, 128, KT * NSL], BF16,
                                    isOutput=False)
    wsb = nc.declare_dram_parameter("wsb", [128, N], F16, isOutput=False)
    y = nc.declare_dram_parameter("y", [M, N], F16, isOutput=True)

    with tile.TileContext(nc) as tc, ExitStack() as ctx:
      pers = ctx.enter_context(tc.tile_pool(name="pers", bufs=1))
      qpool = ctx.enter_context(tc.tile_pool(name="qt", bufs=1))
      xpool = ctx.enter_context(tc.tile_pool(name="xa", bufs=4))
      tpool = ctx.enter_context(tc.tile_pool(name="tmpq", bufs=2))
      qnat = ctx.enter_context(tc.tile_pool(name="qnat", bufs=2))
      wpool = ctx.enter_context(tc.tile_pool(name="wt", bufs=2))
      psum = ctx.enter_context(tc.tile_pool(name="psum", bufs=6, space="PSUM"))
      opool = ctx.enter_context(tc.tile_pool(name="out", bufs=3))
      for rep in range(reps):
        if rep > 0:
            tc.strict_bb_all_engine_barrier()

        # -- software-pipelined x loads (bufs=4): x0 leads the sync ring
        #    ahead of the weight slices; x1-x3 ride the scalar ring. Weight
        #    DMAs are split into 1MB pieces (8KB descriptors) so the DMA
        #    engines' descriptor round-robin shares bandwidth fairly with
        #    the 8KB x descriptors. --
        xas = [None] * MT

        def load_x(mt, eng):
            xas[mt] = xpool.tile([128, K], F16, tag="xa", name=f"xa{mt}")
            eng.dma_start(xas[mt][:], x[mt * 128 : (mt + 1) * 128, :])

        def load_w(ns, wt):
            # 4 pieces of KT*NSL/4 elems (1MB, 8KB/partition contiguous)
            piece = KT * NSL // 4
            for i in range(4):
                nc.sync.dma_start(
                    wt[:, i * piece : (i + 1) * piece],
                    wTr[ns, :, i * piece : (i + 1) * piece],
                )

        if do_quant:
            load_x(0, nc.sync)
        wts = {}
        wts[0] = wpool.tile([128, KT * NSL], BF16, tag="wt", name="wt0")
        wts[1] = wpool.tile([128, KT * NSL], BF16, tag="wt", name="wt1")
        if do_mm:
            load_w(0, wts[0])
            load_w(1, wts[1])
        wsb_sb = pers.tile([128, N], F16)
        if do_quant:
            load_x(1, nc.scalar)
            load_x(2, nc.scalar)
            if MT > 3:
                load_x(3, nc.scalar)
        nc.scalar.dma_start(wsb_sb[:], wsb[:])

        # -- PE warmup: dummy matmuls on zeroed SBUF from t~0 so HAM is at
        #    2.4 GHz (and stays there) when the first real matmul issues --
        if do_mm and WARM > 0:
            warm = pers.tile([128, NSL], BF16)
            nc.vector.memset(warm[:], 0.0)
            wps = psum.tile([128, NSL], F32, tag="pt")
            for _ in range(WARM):
                nc.tensor.matmul(wps[:], warm[:, 0:128], warm[:],
                                 start=True, stop=True)

        am = pers.tile([128, MT], F32)
        scales = pers.tile([128, MT], F32)
        invs = pers.tile([128, MT], F32)
        # m-tile-major so each m-tile's [KT,128] block is contiguous; chunked
        # xbar transposes write qT[p, mt, kt, m] = q[mt*128+m, kt*128+p]
        qT = qpool.tile([128, MT, KT, 128], BF16)

        if not do_quant:
            nc.vector.memset(scales[:], 1.0)
            if do_mm:
                nc.vector.memset(qT[:], 1.0)
        # ---- phase A/B: per m-tile absmax, scales, quantize, transpose ----
        # tile_wait_until = logical priority: keeps the list scheduler from
        # hoisting a later tile's reduce ahead of this tile's scale/quant
        # chain on the in-order DVE queue.
        for mt in range(MT if do_quant else 0):
          with tc.tile_wait_until(mt * 0.005):
            xa = xas[mt]
            nc.vector.tensor_reduce(
                am[:, mt : mt + 1],
                xa[:],
                axis=mybir.AxisListType.X,
                op=mybir.AluOpType.max,
                apply_absolute_value=True,
            )
            # scale = max(absmax/127, 1e-8); inv = 1/scale
            nc.vector.tensor_scalar(
                scales[:, mt : mt + 1],
                am[:, mt : mt + 1],
                1.0 / QMAX,
                1e-8,
                mybir.AluOpType.mult,
                mybir.AluOpType.max,
            )
            nc.vector.reciprocal(invs[:, mt : mt + 1], scales[:, mt : mt + 1])

            qn = qnat.tile([128, K], BF16, tag="qn")
            for kc in range(NQ):
                sl = slice(kc * QCH, (kc + 1) * QCH)
                tmpq = tpool.tile([128, QCH], F32, tag="tmpq")
                # tmpq = x*inv + MAGIC  (fp32; rounds to integer at +MAGIC)
                nc.gpsimd.tensor_scalar(
                    tmpq[:],
                    xa[:, sl],
                    invs[:, mt : mt + 1],
                    MAGIC,
                    mybir.AluOpType.mult,
                    mybir.AluOpType.add,
                )
                # qn = tmpq - MAGIC  (exact; integer-valued, exact in bf16)
                nc.scalar.activation(
                    qn[:, sl],
                    tmpq[:],
                    mybir.ActivationFunctionType.Copy,
                    bias=-MAGIC,
                )
                # chunk xbar transpose [128m, 1024k] -> [128k, KTC, 128m]:
                # the first MM chain can start after chunk 0 lands
                nc.sync.dma_start_transpose(
                    qT[:, mt, kc * KTC : (kc + 1) * KTC], qn[:, sl]
                )
            # next x tile load rides whichever ring frees first; emitted
            # here so its buffer-free wait never blocks a transpose inject
            if mt + 4 < MT:
                load_x(mt + 4, nc.sync if (mt % 2 == 0) else nc.scalar)

        if not do_mm:
            ot0 = opool.tile([128, NSL], F16, tag="ot")
            nc.vector.memset(ot0[:], 0.0)
            nc.sync.dma_start(y[0:128, 0:NSL], ot0[:])
        # ---- phase C: matmul + fused dequant ----
        # First two n-slices interleaved per m-tile (both weight slices are
        # resident): each new qT tile unlocks 2 chains during pipeline fill.
        # (0,0),(0,1) lead so the first chains only need wt0 (wt1's DMA
        # finishes ~30us in).
        ILV = 4  # m-tiles processed (ns0,ns1)-interleaved
        sched = [(0, 0), (0, 1), (1, 0), (1, 1)]
        for mt in range(2, ILV):
            sched += [(0, mt), (1, mt)]
        sched += [(0, mt) for mt in range(ILV, MT)]
        sched += [(1, mt) for mt in range(ILV, MT)]
        for ns in range(2, NS):
            sched += [(ns, mt) for mt in range(MT)]

        for ns, mt in (sched if do_mm else []):
            nsl = slice(ns * NSL, (ns + 1) * NSL)
            if ns not in wts:
                wt = wpool.tile([128, KT * NSL], BF16, tag="wt",
                                name=f"wt{ns}")
                load_w(ns, wt)
                wts[ns] = wt
            wt = wts[ns]
            pt = psum.tile([128, NSL], F32, tag="pt")
            for kt in range(KT):
                nc.tensor.matmul(
                    pt[:],
                    qT[:, mt, kt, :],
                    wt[:, kt * NSL : (kt + 1) * NSL],
                    start=(kt == 0),
                    stop=(kt == KT - 1),
                )
            # ot = (psum * scale[m]) * wscale[n] -> fp16, one DVE op
            ot = opool.tile([128, NSL], F16, tag="ot")
            nc.vector.scalar_tensor_tensor(
                ot[:],
                pt[:],
                scales[:, mt : mt + 1],
                wsb_sb[:, nsl],
                mybir.AluOpType.mult,
                mybir.AluOpType.mult,
            )
            nc.scalar.dma_start(y[mt * 128 : (mt + 1) * 128, nsl], ot[:])

    nc.finalize()  # Bacc.compile(): reg alloc, wait-splitting, event sems
    return nc


def prep_inputs(x, weight, weight_scales, n_cores=8, NSL=512):
    """Host-side shard/layout prep. Returns (in_maps, out_assembler)."""
    B, S, D_in = x.shape
    D_out = weight.shape[0]
    M_total = B * S
    Mc = M_total // n_cores
    KT = D_in // 128
    NS = D_out // NSL

    xf = np.ascontiguousarray(np.asarray(x).reshape(M_total, D_in))
    w = np.asarray(weight)
    wT = w.T.astype(np.float32).astype(ml_dtypes.bfloat16)  # [K, N] exact
    # n-slice-major, partition-contiguous: wTr[ns, p, kt*NSL + j] =
    # wT[kt*128 + p, ns*NSL + j]
    wTr = np.ascontiguousarray(
        wT.reshape(KT, 128, NS, NSL).transpose(2, 1, 0, 3).reshape(
            NS, 128, KT * NSL
        )
    )
    ws = np.asarray(weight_scales).astype(np.float16)
    wsb = np.ascontiguousarray(np.broadcast_to(ws[None, :], (128, D_out)))

    in_maps = [
        {"x": xf[c * Mc : (c + 1) * Mc], "wTr": wTr, "wsb": wsb}
        for c in range(n_cores)
    ]

    def assemble(results):
        return np.concatenate(
            [np.asarray(results[c]["y"]) for c in range(n_cores)], axis=0
        ).reshape(B, S, D_out).astype(np.float16)

    return in_maps, assemble


def kernel(x, weight, weight_scales):
    from concourse.bass_utils import run_bass_kernel_spmd

    n_cores = 8
    B, S, D_in = x.shape
    D_out = weight.shape[0]
    Mc = (B * S) // n_cores

    nc = build_nc(M=Mc, K=D_in, N=D_out)
    in_maps, assemble = prep_inputs(x, weight, weight_scales, n_cores)
    res = run_bass_kernel_spmd(nc, in_maps, list(range(n_cores)))
    return assemble(res.results)


if __name__ == "__main__":
    np.random.seed(0)
    x = np.random.randn(4, 2048, 4096).astype(np.float16)
    w = np.random.randint(-127, 127, (4096, 4096)).astype(np.int8)
    ws = (np.random.rand(4096).astype(np.float32) * 0.01 + 1e-4).astype(np.float16)
    y = kernel(x, w, ws)
    print(y.shape, y.dtype)
